# revision 22
# baseline (speedup 1.0000x reference)
"""Trainium2 Bass kernel for nn_DecoderLayer (self-attn + cross-attn + per-head FFN).

Sharding: pure data-parallel over (batch, query-half): 8 cores = 4 batches x 2
query-halves of 1024 rows each. K/V sources (full x[b] / cross[b]) are
replicated per batch pair; no cross-core communication. The kv sequence is
host-rolled so each core's query rows start at position 0 (softmax is
permutation-invariant over kv positions).

Layouts inside a core:
  - activations position-major [q=128part, d] for residual/LayerNorm
  - projections feature-major via host-pretransposed weights/inputs
  - scores S.T [kpos=128part, q] per head; softmax sums via a ones-column
    appended to V ([V|1] stationary -> AV matmul emits O rows + sum row)
  - per-head FFN with row-packed (tile_position) K=64 matmuls
All matmul operands bf16 (PSUM accumulation fp32); LN/residual math fp32.
"""

import numpy as np
import ml_dtypes
from contextlib import ExitStack

import concourse.bass as bass
import concourse.mybir as mybir
import concourse.tile as tile
from concourse import bacc
from concourse.bass_utils import run_bass_kernel_spmd
from concourse.masks import make_identity

F32 = mybir.dt.float32
BF16 = mybir.dt.bfloat16
NP_BF16 = ml_dtypes.bfloat16

B, L, D, H, HD, FF = 4, 2048, 512, 8, 64, 2048
R = 1024          # query rows per core
NCORES = 8
LN_EPS = 1e-5
KT = D // 128      # 4 feature tiles
QT = R // 128      # 8 query tiles per core
QC = R // 512      # 2 query chunks of 512
KPT = L // 128     # 16 kv-position tiles
FT = FF // 128     # 16 ffn tiles
Exp = mybir.ActivationFunctionType.Exp
Relu = mybir.ActivationFunctionType.Relu
Sqrt = mybir.ActivationFunctionType.Sqrt
Sub = mybir.AluOpType.subtract
Mult = mybir.AluOpType.mult
Add = mybir.AluOpType.add
Max = mybir.AluOpType.max

_CACHE = {}
PACE = True


def _build(flags):
    """Build + compile the per-core Bass program. flags: dict of bools
    (bias_q/k/v/o per block, ln identity, ffn biases) controlling skipped ops."""
    nc = bacc.Bacc("TRN2", target_bir_lowering=False, debug=False)

    dram = {}

    def din(name, shape, dt):
        dram[name] = nc.dram_tensor(name, shape, dt, kind="ExternalInput").ap()
        return dram[name]

    xq_d = din("xq", [R, D], F32)
    xkvT_d = din("xkvT", [D, L], BF16)
    ckvT_d = din("ckvT", [D, L], BF16)
    w_d = {p: din(f"{p}_w", [D, 4, D], BF16) for p in ("self", "cross")}
    w1T_d = din("w1T", [4, 128, FF], BF16)
    w2T_d = din("w2T", [H, FF, HD], BF16)
    bqkv_d = {}
    for p in ("self", "cross"):
        for i, nm in enumerate(("q", "k", "v")):
            if not flags[f"{p}_b{nm}_zero"]:
                bqkv_d[(p, nm)] = din(f"{p}_b{nm}", [D], F32)
        if not flags[f"{p}_bo_zero"]:
            bqkv_d[(p, "o")] = din(f"{p}_bo", [D], F32)
    ln_d = {}
    for i in (1, 2, 3):
        if not flags[f"ln{i}_id"]:
            ln_d[i] = (din(f"ln{i}_g", [D], F32), din(f"ln{i}_b", [D], F32))
    if not flags["ffb1_zero"]:
        ffb1_d = din("ffb1", [H, FF], F32)
    if not flags["ffb2_zero"]:
        ffb2_d = din("ffb2", [D], F32)
    out_d = nc.dram_tensor("out", [R, D], F32, kind="ExternalOutput").ap()

    with ExitStack() as ctx:
        tc = ctx.enter_context(tile.TileContext(nc))
        ec = ctx.enter_context
        constp = ec(tc.tile_pool(name="const", bufs=1))
        residp = ec(tc.tile_pool(name="resid", bufs=2))
        xtp = ec(tc.tile_pool(name="xt", bufs=2))
        wtp = ec(tc.tile_pool(name="wt", bufs=2))
        w2p = ec(tc.tile_pool(name="w2p", bufs=1))
        qkvp = ec(tc.tile_pool(name="qkv", bufs=1))
        expp = ec(tc.tile_pool(name="expp", bufs=2))
        hpp = ec(tc.tile_pool(name="hp", bufs=2))
        smallp = ec(tc.tile_pool(name="small", bufs=1))
        tpool = ec(tc.tile_pool(name="tp", bufs=4))
        rbp = ec(tc.tile_pool(name="rbp", bufs=1))
        mvp = ec(tc.tile_pool(name="mvp", bufs=2))
        ps_mm = ec(tc.tile_pool(name="ps_mm", bufs=2, space="PSUM"))
        ps_sc = ec(tc.tile_pool(name="ps_sc", bufs=2, space="PSUM"))
        ps_av = ec(tc.tile_pool(name="ps_av", bufs=2, space="PSUM"))
        drp = ec(tc.tile_pool(name="drp", bufs=2, space="DRAM"))
        if True:
            # ---- constants ----
            ident = constp.tile([128, 128], F32, tag="ident")
            make_identity(nc, ident)
            ident16 = constp.tile([128, 128], BF16, tag="ident16")
            make_identity(nc, ident16)
            eps_s = constp.tile([128, 1], F32, tag="eps")
            nc.vector.memset(eps_s, LN_EPS)

            def bcast_load(dvec, tag):
                t = constp.tile([128, D], F32, tag=tag)
                src = bass.AP(tensor=dvec.tensor, offset=dvec.offset,
                              ap=[[0, 128]] + list(dvec.ap))
                nc.sync.dma_start(out=t, in_=src)
                return t

            def ppart_load(dvec, tag):
                # [D] -> [128, KT] per-partition layout (feature-major bias)
                t = constp.tile([128, KT], F32, tag=tag)
                nc.sync.dma_start(out=t, in_=dvec.rearrange("(t p) -> p t", p=128))
                return t

            bq_s = {}
            for (p, nm), dv in bqkv_d.items():
                if nm == "o":
                    bq_s[(p, "o")] = bcast_load(dv, f"{p}_bo_s")
                else:
                    bq_s[(p, nm)] = ppart_load(dv, f"{p}_b{nm}_s")
            ln_s = {}
            for i, (g, b) in ln_d.items():
                ln_s[i] = (bcast_load(g, f"ln{i}g_s"), bcast_load(b, f"ln{i}b_s"))
            if not flags["ffb1_zero"]:
                ffb1_s = constp.tile([128, H, FT], F32, tag="ffb1_s")
                nc.sync.dma_start(out=ffb1_s,
                                  in_=ffb1_d.rearrange("h (t p) -> p h t", p=128))
            if not flags["ffb2_zero"]:
                ffb2_s = bcast_load(ffb2_d, "ffb2_s")

            # ---- bulk input loads ----
            xq_s = residp.tile([128, QT, D], F32, tag="resid")
            nc.sync.dma_start(out=xq_s, in_=xq_d.rearrange("(t p) d -> p t d", p=128))
            xkvT_s = xtp.tile([128, KT, L], BF16, tag="xt")
            xkvT_r = xkvT_d.rearrange("(t p) l -> p t l", p=128)
            w_s = {}
            for p in ("self", "cross"):
                w_s[p] = wtp.tile([128, KT, 4, D], BF16, tag="wt", name=f"w_{p}")
            for kt in range(KT):
                nc.sync.dma_start(out=w_s["self"][:, kt, :, :],
                                  in_=w_d["self"].rearrange("(t p) w m -> p t w m", p=128)[:, kt, :, :])
                nc.sync.dma_start(out=xkvT_s[:, kt, :], in_=xkvT_r[:, kt, :])
            ckvT_s = xtp.tile([128, KT, L], BF16, tag="xt")
            nc.sync.dma_start(out=ckvT_s, in_=ckvT_d.rearrange("(t p) l -> p t l", p=128))
            nc.sync.dma_start(out=w_s["cross"],
                              in_=w_d["cross"].rearrange("(t p) w m -> p t w m", p=128))
            w2T_s = w2p.tile([128, H, FT, HD], BF16, tag="w2")
            nc.sync.dma_start(out=w2T_s,
                              in_=w2T_d.rearrange("h (t p) d -> p h t d", p=128))

            def layernorm_rows(qts, lnidx, src_fn, dst_ap_fn, post_fn=None):
                """LN over a group of q-tiles. src_fn(qt)->SBUF f32 [128, D] AP;
                normalized output written to dst_ap_fn(qt); post_fn(qt) after."""
                gsz = len(qts)
                mv = mvp.tile([128, gsz, 2], F32, tag="mv")
                srcs = []
                for i, qt in enumerate(qts):
                    sx = src_fn(qt)
                    srcs.append(sx)
                    stats = smallp.tile([128, 6], F32, tag="stats")
                    nc.vector.bn_stats(out=stats, in_=sx)
                    nc.vector.bn_aggr(out=mv[:, i, :], in_=stats)
                rstd = smallp.tile([128, gsz], F32, tag="rstd")
                nc.scalar.activation(out=rstd, in_=mv[:, :, 1],
                                     func=Sqrt, bias=eps_s, scale=1.0)
                nc.vector.reciprocal(out=rstd, in_=rstd)
                for i, qt in enumerate(qts):
                    o = dst_ap_fn(qt)
                    nc.vector.tensor_scalar(out=o, in0=srcs[i],
                                            scalar1=mv[:, i, 0:1],
                                            scalar2=rstd[:, i:i + 1],
                                            op0=Sub, op1=Mult)
                    if lnidx in ln_s:
                        g_t, b_t = ln_s[lnidx]
                        nc.vector.tensor_mul(out=o, in0=o, in1=g_t)
                        nc.vector.tensor_add(out=o, in0=o, in1=b_t)
                    if post_fn is not None:
                        post_fn(qt)

            def attn_block(pre, qsrcT_s, qcols, kvT_s, resid_s, lnidx):
                """One attention block. Returns (x_next_s f32 [128,QT,D],
                xnextT_s bf16 [128,KT,R])."""
                wts = w_s[pre]
                # Q projection (feature-major) [128, KT, R]
                qt_s = qkvp.tile([128, KT, R], BF16, tag="qt")
                for mt in range(KT):
                    for qc in range(QC):
                        ps = ps_mm.tile([128, 512], F32, tag="mm")
                        for kt in range(KT):
                            nc.tensor.matmul(
                                ps, wts[:, kt, 0, mt * 128:(mt + 1) * 128],
                                qsrcT_s[:, kt, qcols + qc * 512: qcols + qc * 512 + 512],
                                start=(kt == 0), stop=(kt == KT - 1))
                        dst = qt_s[:, mt, qc * 512:(qc + 1) * 512]
                        if (pre, "q") in bq_s:
                            nc.vector.tensor_scalar_add(out=dst, in0=ps,
                                                        scalar1=bq_s[(pre, "q")][:, mt:mt + 1])
                        else:
                            nc.vector.tensor_copy(out=dst, in_=ps)
                # K projection (feature-major) [128, KT, L]
                kt_s = qkvp.tile([128, KT, L], BF16, tag="kt", bufs=2)
                for mt in range(KT):
                    for lc in range(L // 512):
                        ps = ps_mm.tile([128, 512], F32, tag="mm")
                        for kt in range(KT):
                            nc.tensor.matmul(
                                ps, wts[:, kt, 1, mt * 128:(mt + 1) * 128],
                                kvT_s[:, kt, lc * 512:(lc + 1) * 512],
                                start=(kt == 0), stop=(kt == KT - 1))
                        dst = kt_s[:, mt, lc * 512:(lc + 1) * 512]
                        if (pre, "k") in bq_s:
                            nc.vector.tensor_scalar_add(out=dst, in0=ps,
                                                        scalar1=bq_s[(pre, "k")][:, mt:mt + 1])
                        else:
                            nc.vector.tensor_copy(out=dst, in_=ps)
                # V position-major with ones column [128, KPT, H, HD+1]
                vext_s = qkvp.tile([128, KPT, H, HD + 1], BF16, tag="vext")
                nc.vector.memset(vext_s[:, :, :, HD:HD + 1], 1.0)
                for kpt in range(KPT):
                    ps = ps_mm.tile([128, 512], F32, tag="mm")
                    for kt in range(KT):
                        nc.tensor.matmul(
                            ps, kvT_s[:, kt, kpt * 128:(kpt + 1) * 128],
                            wts[:, kt, 2, :],
                            start=(kt == 0), stop=(kt == KT - 1))
                    dst = vext_s[:, kpt, :, 0:HD]
                    psv = ps.rearrange("p (h d) -> p h d", h=H)
                    if (pre, "v") in bq_s:
                        nc.vector.tensor_add(
                            out=dst, in0=psv,
                            in1=bq_s[(pre, "v")].rearrange("p (h d) -> p h d", h=H))
                    else:
                        nc.vector.tensor_copy(out=dst, in_=psv)

                # attention, qc-outer for cross-stage pipelining;
                # head pairs interleaved so tile_position row-groups overlap
                ot_s = qkvp.tile([128, KT, R], BF16, tag="ot")
                x_next = residp.tile([128, QT, D], F32, tag="resid")
                xnextT = xtp.tile([128, KT, R], BF16, tag="xt")
                for qc in range(QC):
                    for hp in range(H // 2):
                        dt_h = hp
                        if PACE:
                            pdum = ps_mm.tile([128, 512], F32, tag="mm",
                                              name="pdum")
                        pavs = []
                        for par in range(2):
                            pav = ps_av.tile([HD + 1, 512], F32, tag="av",
                                             name=f"pav{par}")
                            pavs.append(pav)
                        for pr in range(8):
                            e_s = expp.tile([128, 2, 2, 512], BF16, tag="expS")
                            scs = []
                            for par in range(2):
                                pb = 64 * par
                                sc = ps_sc.tile([128, 2, 512], F32, tag="sc",
                                                name=f"sc{par}")
                                scs.append(sc)
                                for t in range(2):
                                    kpt = pr * 2 + t
                                    nc.tensor.matmul(
                                        sc[:, t, :],
                                        kt_s[pb:pb + 64, dt_h, kpt * 128:(kpt + 1) * 128],
                                        qt_s[pb:pb + 64, dt_h, qc * 512:(qc + 1) * 512],
                                        start=True, stop=True,
                                        tile_position=(pb, 0))
                            for par in range(2):
                                nc.scalar.activation(
                                    out=e_s[:, par, :, :], in_=scs[par], func=Exp)
                            for k in range(2):
                                kpt = pr * 2 + k
                                for par in range(2):
                                    h = 2 * hp + par
                                    nc.tensor.matmul(
                                        pavs[par], vext_s[:, kpt, h, :],
                                        e_s[:, par, k, :],
                                        start=(kpt == 0), stop=(kpt == KPT - 1))
                            if PACE:
                                nc.tensor.matmul(
                                    pdum[:, 0:256], ident16,
                                    w2T_s.rearrange("p h t d -> p (h t d)")[:, 0:256],
                                    start=(pr == 0), stop=(pr == 7))
                        # drain pav fast: raw O rows + sums row, then normalize
                        for par in range(2):
                            h = 2 * hp + par
                            pb = 64 * par
                            pav = pavs[par]
                            oslice = ot_s[pb:pb + 64, dt_h, qc * 512:(qc + 1) * 512]
                            nc.vector.tensor_copy(out=oslice, in_=pav[0:HD, :])
                            srow = smallp.tile([1, 512], F32, tag="srow", bufs=1)
                            nc.vector.tensor_copy(out=srow, in_=pav[HD:HD + 1, :])
                            dtmp = drp.tile([1, 512], F32, tag="dtmp")
                            nc.sync.dma_start(out=dtmp, in_=srow)
                            rb = rbp.tile([128, 512], F32, tag="rb")
                            nc.sync.dma_start(
                                out=rb,
                                in_=bass.AP(tensor=dtmp.tensor, offset=dtmp.offset,
                                            ap=[[0, 128]] + list(dtmp.ap[1:])))
                            nc.vector.reciprocal_approx_fast(out=rb, in_=rb)
                            nc.vector.tensor_mul(out=oslice, in0=oslice,
                                                 in1=rb[pb:pb + 64, :])

                    # per-qc: O-projection + bias + residual + LN + transpose
                    def osrc(qt):
                        ps = ps_mm.tile([128, 512], F32, tag="mm")
                        for dt in range(KT):
                            nc.tensor.matmul(ps, ot_s[:, dt, qt * 128:(qt + 1) * 128],
                                             wts[:, dt, 3, :],
                                             start=(dt == 0), stop=(dt == KT - 1))
                        t = tpool.tile([128, D], F32, tag="t")
                        nc.vector.tensor_add(out=t, in0=ps, in1=resid_s[:, qt, :])
                        if (pre, "o") in bq_s:
                            nc.vector.tensor_add(out=t, in0=t, in1=bq_s[(pre, "o")])
                        return t

                    if PACE:
                        pdum3 = ps_mm.tile([128, 512], F32, tag="mm", name="pdum3")

                    def post(qt):
                        for dt in range(KT):
                            ptr = ps_mm.tile([128, 128], F32, tag="mm")
                            nc.tensor.transpose(
                                ptr, x_next[:, qt, dt * 128:(dt + 1) * 128], ident)
                            nc.vector.tensor_copy(
                                out=xnextT[:, dt, qt * 128:(qt + 1) * 128], in_=ptr)
                        if PACE:
                            nc.tensor.matmul(
                                pdum3[:, 0:256], ident16,
                                w2T_s.rearrange("p h t d -> p (h t d)")[:, 0:256],
                                start=(qt % 4 == 0), stop=(qt % 4 == 3))

                    layernorm_rows([qc * 4 + i for i in range(4)], lnidx, osrc,
                                   lambda qt: x_next[:, qt, :], post)
                return x_next, xnextT

            x1_s, x1T_s = attn_block("self", xkvT_s, 0, xkvT_s, xq_s, 1)
            x2_s, x2T_s = attn_block("cross", x1T_s, 0, ckvT_s, x1_s, 2)

            # ---- per-head FFN: head-pair packed, feature-major y,
            # transpose fused with residual accumulation into x2_s ----
            w1T_s = qkvp.tile([128, 4, FF], BF16, tag="kt", bufs=2)
            nc.sync.dma_start(out=w1T_s, in_=w1T_d.rearrange("hp p f -> p hp f"))
            for qc in range(QC):
                for hp in range(H // 2):
                    pyt = ps_av.tile([128, 512], F32, tag="av")
                    if PACE:
                        pdum2 = ps_mm.tile([128, 512], F32, tag="mm",
                                           name="pdum2")
                    for chunk in range(8):
                        hT = hpp.tile([128, 4, 512], BF16, tag="hT")
                        for f in range(2):
                            ft = chunk * 2 + f
                            for par in range(2):
                                h = 2 * hp + par
                                pb = 64 * par
                                ps = ps_mm.tile([128, 512], F32, tag="mm")
                                nc.tensor.matmul(
                                    ps, w1T_s[pb:pb + 64, hp, ft * 128:(ft + 1) * 128],
                                    x2T_s[pb:pb + 64, hp, qc * 512:(qc + 1) * 512],
                                    start=True, stop=True, tile_position=(pb, 0))
                                dst = hT[:, 2 * f + par, :]
                                if not flags["ffb1_zero"]:
                                    if par == 0:
                                        nc.scalar.activation(out=dst, in_=ps,
                                                             func=Relu,
                                                             bias=ffb1_s[:, h, ft:ft + 1])
                                    else:
                                        nc.vector.tensor_scalar(
                                            out=dst, in0=ps,
                                            scalar1=ffb1_s[:, h, ft:ft + 1],
                                            scalar2=0.0, op0=Add, op1=Max)
                                else:
                                    if par == 0:
                                        nc.scalar.activation(out=dst, in_=ps,
                                                             func=Relu)
                                    else:
                                        nc.vector.tensor_scalar_max(out=dst,
                                                                    in0=ps,
                                                                    scalar1=0.0)
                        # FFN2: col-packed pair, accumulate into pyt [128, 512]
                        for f in range(2):
                            ft = chunk * 2 + f
                            for par in range(2):
                                h = 2 * hp + par
                                nc.tensor.matmul(
                                    pyt[64 * par:64 * par + 64, :],
                                    w2T_s[:, h, ft, :],
                                    hT[:, 2 * f + par, :],
                                    start=(ft == 0), stop=(ft == FT - 1),
                                    tile_position=(0, 64 * par))
                        if PACE:
                            nc.tensor.matmul(
                                pdum2[:, 0:256], ident16,
                                w1T_s.rearrange("p a b -> p (a b)")[:, 0:256].bitcast(BF16),
                                start=(chunk == 0), stop=(chunk == 7))

                    # drain yT pair: transpose + residual-accumulate into x2_s
                    yT = hpp.tile([128, 512], BF16, tag="yT", bufs=2)
                    if PACE:
                        pass
                    nc.vector.tensor_copy(out=yT, in_=pyt)
                    for qt4 in range(4):
                        qt = qc * 4 + qt4
                        ptr = ps_mm.tile([128, 128], BF16, tag="mm")
                        nc.tensor.transpose(
                            ptr, yT[:, qt4 * 128:(qt4 + 1) * 128], ident16)
                        sl = x2_s[:, qt, hp * 128:(hp + 1) * 128]
                        nc.vector.tensor_add(out=sl, in0=sl, in1=ptr)

            # ---- LN3 + output ----
            def src3(qt):
                if not flags["ffb2_zero"]:
                    nc.vector.tensor_add(out=x2_s[:, qt, :], in0=x2_s[:, qt, :],
                                         in1=ffb2_s)
                return x2_s[:, qt, :]

            out_r = out_d.rearrange("(t p) d -> p t d", p=128)
            for g0 in (0, 4):
                outs = {}

                def dst3(qt):
                    o = tpool.tile([128, D], F32, tag="t")
                    outs[qt] = o
                    return o

                def post3(qt):
                    nc.sync.dma_start(out=out_r[:, qt, :], in_=outs[qt])

                layernorm_rows([g0 + i for i in range(4)], 3, src3, dst3, post3)

    nc.compile()
    return nc


def kernel(x, cross, params):
    x = np.asarray(x, dtype=np.float32)
    cross = np.asarray(cross, dtype=np.float32)
    p = {k: np.asarray(v) for k, v in params.items()}

    flags = {}
    for pre in ("self", "cross"):
        for nm in ("q", "k", "v", "o"):
            flags[f"{pre}_b{nm}_zero"] = not np.any(p[f"{pre}_b{nm}"])
    for i in (1, 2, 3):
        flags[f"ln{i}_id"] = (np.all(p[f"ln{i}_g"] == 1.0)
                              and not np.any(p[f"ln{i}_b"]))
    flags["ffb1_zero"] = not np.any(p["ff_b1"])
    flags["ffb2_zero"] = not np.any(p["ff_b2"])

    key = tuple(sorted(flags.items()))
    if key not in _CACHE:
        _CACHE[key] = _build(flags)
    nc = _CACHE[key]

    def wpack(pre):
        mats = []
        for nm, scale in (("q", 0.125), ("k", 1.0), ("v", 1.0), ("o", 1.0)):
            mats.append((p[f"{pre}_w{nm}"].T * scale).astype(NP_BF16))
        return np.ascontiguousarray(np.stack(mats, axis=1))  # [D, 4, D]

    shared = {
        "self_w": wpack("self"),
        "cross_w": wpack("cross"),
        "w1T": np.ascontiguousarray(
            p["ff_w1"].transpose(0, 2, 1).reshape(4, 128, FF)).astype(NP_BF16),
        "w2T": np.ascontiguousarray(p["ff_w2"].transpose(0, 2, 1)).astype(NP_BF16),
    }
    for pre in ("self", "cross"):
        for nm in ("q", "k", "v", "o"):
            if not flags[f"{pre}_b{nm}_zero"]:
                v = p[f"{pre}_b{nm}"].astype(np.float32)
                if nm == "q":
                    v = v * 0.125
                shared[f"{pre}_b{nm}"] = v
    for i in (1, 2, 3):
        if not flags[f"ln{i}_id"]:
            shared[f"ln{i}_g"] = p[f"ln{i}_g"].astype(np.float32)
            shared[f"ln{i}_b"] = p[f"ln{i}_b"].astype(np.float32)
    if not flags["ffb1_zero"]:
        shared["ffb1"] = p["ff_b1"].astype(np.float32)
    if not flags["ffb2_zero"]:
        shared["ffb2"] = p["ff_b2"].reshape(D).astype(np.float32)

    in_maps = []
    ckvT_cache = {}
    for c in range(NCORES):
        b, half = c // 2, c % 2
        xr = np.roll(x[b], -R * half, axis=0)
        if b not in ckvT_cache:
            ckvT_cache[b] = np.ascontiguousarray(cross[b].T).astype(NP_BF16)
        m = dict(shared)
        m["xq"] = np.ascontiguousarray(xr[:R])
        m["xkvT"] = np.ascontiguousarray(xr.T).astype(NP_BF16)
        m["ckvT"] = ckvT_cache[b]
        in_maps.append(m)

    res = run_bass_kernel_spmd(nc, in_maps, core_ids=list(range(NCORES)))
    kernel._last_results = res

    y = np.empty((B, L, D), np.float32)
    for c in range(NCORES):
        b, half = c // 2, c % 2
        y[b, R * half:R * (half + 1)] = res.results[c]["out"]
    return y


# revision 23
# speedup vs baseline: 1.0464x; 1.0464x over previous
"""Trainium2 Bass kernel for nn_DecoderLayer (self-attn + cross-attn + per-head FFN).

Sharding: pure data-parallel over (batch, query-half): 8 cores = 4 batches x 2
query-halves of 1024 rows each. K/V sources (full x[b] / cross[b]) are
replicated per batch pair; no cross-core communication. The kv sequence is
host-rolled so each core's query rows start at position 0 (softmax is
permutation-invariant over kv positions).

Layouts inside a core:
  - activations position-major [q=128part, d] for residual/LayerNorm
  - projections feature-major via host-pretransposed weights/inputs
  - scores S.T [kpos=128part, q] per head; softmax sums via a ones-column
    appended to V ([V|1] stationary -> AV matmul emits O rows + sum row)
  - per-head FFN with row-packed (tile_position) K=64 matmuls
All matmul operands bf16 (PSUM accumulation fp32); LN/residual math fp32.
"""

import numpy as np
import ml_dtypes
from contextlib import ExitStack

import concourse.bass as bass
import concourse.mybir as mybir
import concourse.tile as tile
from concourse import bacc
from concourse.bass_utils import run_bass_kernel_spmd
from concourse.masks import make_identity

F32 = mybir.dt.float32
BF16 = mybir.dt.bfloat16
NP_BF16 = ml_dtypes.bfloat16

B, L, D, H, HD, FF = 4, 2048, 512, 8, 64, 2048
R = 1024          # query rows per core
NCORES = 8
LN_EPS = 1e-5
KT = D // 128      # 4 feature tiles
QT = R // 128      # 8 query tiles per core
QC = R // 512      # 2 query chunks of 512
KPT = L // 128     # 16 kv-position tiles
FT = FF // 128     # 16 ffn tiles
Exp = mybir.ActivationFunctionType.Exp
Relu = mybir.ActivationFunctionType.Relu
Sqrt = mybir.ActivationFunctionType.Sqrt
Sub = mybir.AluOpType.subtract
Mult = mybir.AluOpType.mult
Add = mybir.AluOpType.add
Max = mybir.AluOpType.max

_CACHE = {}
PACE = False


def _build(flags):
    """Build + compile the per-core Bass program. flags: dict of bools
    (bias_q/k/v/o per block, ln identity, ffn biases) controlling skipped ops."""
    nc = bacc.Bacc("TRN2", target_bir_lowering=False, debug=False)

    dram = {}

    def din(name, shape, dt):
        dram[name] = nc.dram_tensor(name, shape, dt, kind="ExternalInput").ap()
        return dram[name]

    xq_d = din("xq", [R, D], F32)
    xkvT_d = din("xkvT", [D, L], BF16)
    ckvT_d = din("ckvT", [D, L], BF16)
    w_d = {p: din(f"{p}_w", [D, 4, D], BF16) for p in ("self", "cross")}
    w1T_d = din("w1T", [4, 128, FF], BF16)
    w2T_d = din("w2T", [H, FF, HD], BF16)
    bqkv_d = {}
    for p in ("self", "cross"):
        for i, nm in enumerate(("q", "k", "v")):
            if not flags[f"{p}_b{nm}_zero"]:
                bqkv_d[(p, nm)] = din(f"{p}_b{nm}", [D], F32)
        if not flags[f"{p}_bo_zero"]:
            bqkv_d[(p, "o")] = din(f"{p}_bo", [D], F32)
    ln_d = {}
    for i in (1, 2, 3):
        if not flags[f"ln{i}_id"]:
            ln_d[i] = (din(f"ln{i}_g", [D], F32), din(f"ln{i}_b", [D], F32))
    if not flags["ffb1_zero"]:
        ffb1_d = din("ffb1", [H, FF], F32)
    if not flags["ffb2_zero"]:
        ffb2_d = din("ffb2", [D], F32)
    out_d = nc.dram_tensor("out", [R, D], F32, kind="ExternalOutput").ap()

    with ExitStack() as ctx:
        tc = ctx.enter_context(tile.TileContext(nc))
        ec = ctx.enter_context
        constp = ec(tc.tile_pool(name="const", bufs=1))
        residp = ec(tc.tile_pool(name="resid", bufs=2))
        xtp = ec(tc.tile_pool(name="xt", bufs=2))
        wtp = ec(tc.tile_pool(name="wt", bufs=2))
        w2p = ec(tc.tile_pool(name="w2p", bufs=1))
        qkvp = ec(tc.tile_pool(name="qkv", bufs=1))
        expp = ec(tc.tile_pool(name="expp", bufs=2))
        hpp = ec(tc.tile_pool(name="hp", bufs=2))
        smallp = ec(tc.tile_pool(name="small", bufs=1))
        tpool = ec(tc.tile_pool(name="tp", bufs=4))
        rbp = ec(tc.tile_pool(name="rbp", bufs=1))
        mvp = ec(tc.tile_pool(name="mvp", bufs=2))
        ps_mm = ec(tc.tile_pool(name="ps_mm", bufs=2, space="PSUM"))
        ps_sc = ec(tc.tile_pool(name="ps_sc", bufs=2, space="PSUM"))
        ps_av = ec(tc.tile_pool(name="ps_av", bufs=2, space="PSUM"))
        drp = ec(tc.tile_pool(name="drp", bufs=2, space="DRAM"))
        if True:
            # ---- constants ----
            ident = constp.tile([128, 128], F32, tag="ident")
            make_identity(nc, ident)
            ident16 = constp.tile([128, 128], BF16, tag="ident16")
            make_identity(nc, ident16)
            eps_s = constp.tile([128, 1], F32, tag="eps")
            nc.vector.memset(eps_s, LN_EPS)

            def bcast_load(dvec, tag):
                t = constp.tile([128, D], F32, tag=tag)
                src = bass.AP(tensor=dvec.tensor, offset=dvec.offset,
                              ap=[[0, 128]] + list(dvec.ap))
                nc.sync.dma_start(out=t, in_=src)
                return t

            def ppart_load(dvec, tag):
                # [D] -> [128, KT] per-partition layout (feature-major bias)
                t = constp.tile([128, KT], F32, tag=tag)
                nc.sync.dma_start(out=t, in_=dvec.rearrange("(t p) -> p t", p=128))
                return t

            bq_s = {}
            for (p, nm), dv in bqkv_d.items():
                if nm == "o":
                    bq_s[(p, "o")] = bcast_load(dv, f"{p}_bo_s")
                else:
                    bq_s[(p, nm)] = ppart_load(dv, f"{p}_b{nm}_s")
            ln_s = {}
            for i, (g, b) in ln_d.items():
                ln_s[i] = (bcast_load(g, f"ln{i}g_s"), bcast_load(b, f"ln{i}b_s"))
            if not flags["ffb1_zero"]:
                ffb1_s = constp.tile([128, H, FT], F32, tag="ffb1_s")
                nc.sync.dma_start(out=ffb1_s,
                                  in_=ffb1_d.rearrange("h (t p) -> p h t", p=128))
            if not flags["ffb2_zero"]:
                ffb2_s = bcast_load(ffb2_d, "ffb2_s")

            # ---- bulk input loads ----
            xq_s = residp.tile([128, QT, D], F32, tag="resid")
            nc.sync.dma_start(out=xq_s, in_=xq_d.rearrange("(t p) d -> p t d", p=128))
            xkvT_s = xtp.tile([128, KT, L], BF16, tag="xt")
            xkvT_r = xkvT_d.rearrange("(t p) l -> p t l", p=128)
            w_s = {}
            for p in ("self", "cross"):
                w_s[p] = wtp.tile([128, KT, 4, D], BF16, tag="wt", name=f"w_{p}")
            for kt in range(KT):
                nc.sync.dma_start(out=w_s["self"][:, kt, :, :],
                                  in_=w_d["self"].rearrange("(t p) w m -> p t w m", p=128)[:, kt, :, :])
                nc.sync.dma_start(out=xkvT_s[:, kt, :], in_=xkvT_r[:, kt, :])
            ckvT_s = xtp.tile([128, KT, L], BF16, tag="xt")
            nc.sync.dma_start(out=ckvT_s, in_=ckvT_d.rearrange("(t p) l -> p t l", p=128))
            nc.sync.dma_start(out=w_s["cross"],
                              in_=w_d["cross"].rearrange("(t p) w m -> p t w m", p=128))
            w2T_s = w2p.tile([128, H, FT, HD], BF16, tag="w2")
            nc.sync.dma_start(out=w2T_s,
                              in_=w2T_d.rearrange("h (t p) d -> p h t d", p=128))

            def layernorm_rows(qts, lnidx, src_fn, dst_ap_fn, post_fn=None):
                """LN over a group of q-tiles. src_fn(qt)->SBUF f32 [128, D] AP;
                normalized output written to dst_ap_fn(qt); post_fn(qt) after."""
                gsz = len(qts)
                mv = mvp.tile([128, gsz, 2], F32, tag="mv")
                srcs = []
                for i, qt in enumerate(qts):
                    sx = src_fn(qt)
                    srcs.append(sx)
                    stats = smallp.tile([128, 6], F32, tag="stats")
                    nc.vector.bn_stats(out=stats, in_=sx)
                    nc.vector.bn_aggr(out=mv[:, i, :], in_=stats)
                rstd = smallp.tile([128, gsz], F32, tag="rstd")
                nc.scalar.activation(out=rstd, in_=mv[:, :, 1],
                                     func=Sqrt, bias=eps_s, scale=1.0)
                nc.vector.reciprocal(out=rstd, in_=rstd)
                for i, qt in enumerate(qts):
                    o = dst_ap_fn(qt)
                    nc.vector.tensor_scalar(out=o, in0=srcs[i],
                                            scalar1=mv[:, i, 0:1],
                                            scalar2=rstd[:, i:i + 1],
                                            op0=Sub, op1=Mult)
                    if lnidx in ln_s:
                        g_t, b_t = ln_s[lnidx]
                        nc.vector.tensor_mul(out=o, in0=o, in1=g_t)
                        nc.vector.tensor_add(out=o, in0=o, in1=b_t)
                    if post_fn is not None:
                        post_fn(qt)

            def attn_block(pre, qsrcT_s, qcols, kvT_s, resid_s, lnidx):
                """One attention block. Returns (x_next_s f32 [128,QT,D],
                xnextT_s bf16 [128,KT,R])."""
                wts = w_s[pre]
                # Q projection (feature-major) [128, KT, R]
                qt_s = qkvp.tile([128, KT, R], BF16, tag="qt")
                for mt in range(KT):
                    for qc in range(QC):
                        ps = ps_mm.tile([128, 512], F32, tag="mm")
                        for kt in range(KT):
                            nc.tensor.matmul(
                                ps, wts[:, kt, 0, mt * 128:(mt + 1) * 128],
                                qsrcT_s[:, kt, qcols + qc * 512: qcols + qc * 512 + 512],
                                start=(kt == 0), stop=(kt == KT - 1))
                        dst = qt_s[:, mt, qc * 512:(qc + 1) * 512]
                        if (pre, "q") in bq_s:
                            nc.vector.tensor_scalar_add(out=dst, in0=ps,
                                                        scalar1=bq_s[(pre, "q")][:, mt:mt + 1])
                        else:
                            nc.vector.tensor_copy(out=dst, in_=ps)
                # K projection (feature-major) [128, KT, L]
                kt_s = qkvp.tile([128, KT, L], BF16, tag="kt", bufs=2)
                for mt in range(KT):
                    for lc in range(L // 512):
                        ps = ps_mm.tile([128, 512], F32, tag="mm")
                        for kt in range(KT):
                            nc.tensor.matmul(
                                ps, wts[:, kt, 1, mt * 128:(mt + 1) * 128],
                                kvT_s[:, kt, lc * 512:(lc + 1) * 512],
                                start=(kt == 0), stop=(kt == KT - 1))
                        dst = kt_s[:, mt, lc * 512:(lc + 1) * 512]
                        if (pre, "k") in bq_s:
                            nc.vector.tensor_scalar_add(out=dst, in0=ps,
                                                        scalar1=bq_s[(pre, "k")][:, mt:mt + 1])
                        else:
                            nc.vector.tensor_copy(out=dst, in_=ps)
                # V position-major with ones column [128, KPT, H, HD+1]
                vext_s = qkvp.tile([128, KPT, H, HD + 1], BF16, tag="vext")
                nc.vector.memset(vext_s[:, :, :, HD:HD + 1], 1.0)
                for kpt in range(KPT):
                    ps = ps_mm.tile([128, 512], F32, tag="mm")
                    for kt in range(KT):
                        nc.tensor.matmul(
                            ps, kvT_s[:, kt, kpt * 128:(kpt + 1) * 128],
                            wts[:, kt, 2, :],
                            start=(kt == 0), stop=(kt == KT - 1))
                    dst = vext_s[:, kpt, :, 0:HD]
                    psv = ps.rearrange("p (h d) -> p h d", h=H)
                    if (pre, "v") in bq_s:
                        nc.vector.tensor_add(
                            out=dst, in0=psv,
                            in1=bq_s[(pre, "v")].rearrange("p (h d) -> p h d", h=H))
                    else:
                        nc.vector.tensor_copy(out=dst, in_=psv)

                # attention, qc-outer for cross-stage pipelining;
                # head pairs interleaved so tile_position row-groups overlap
                ot_s = qkvp.tile([128, KT, R], BF16, tag="ot")
                x_next = residp.tile([128, QT, D], F32, tag="resid")
                xnextT = xtp.tile([128, KT, R], BF16, tag="xt")
                for qc in range(QC):
                    for hp in range(H // 2):
                        dt_h = hp
                        if PACE:
                            pdum = ps_mm.tile([128, 512], F32, tag="mm",
                                              name="pdum")
                        pavs = []
                        for par in range(2):
                            pav = ps_av.tile([HD + 1, 512], F32, tag="av",
                                             name=f"pav{par}")
                            pavs.append(pav)
                        for pr in range(8):
                            e_s = expp.tile([128, 2, 2, 512], BF16, tag="expS")
                            scs = []
                            for par in range(2):
                                pb = 64 * par
                                sc = ps_sc.tile([128, 2, 512], F32, tag="sc",
                                                name=f"sc{par}")
                                scs.append(sc)
                                for t in range(2):
                                    kpt = pr * 2 + t
                                    nc.tensor.matmul(
                                        sc[:, t, :],
                                        kt_s[pb:pb + 64, dt_h, kpt * 128:(kpt + 1) * 128],
                                        qt_s[pb:pb + 64, dt_h, qc * 512:(qc + 1) * 512],
                                        start=True, stop=True,
                                        tile_position=(pb, 0))
                            for par in range(2):
                                nc.scalar.activation(
                                    out=e_s[:, par, :, :], in_=scs[par], func=Exp)
                            for k in range(2):
                                kpt = pr * 2 + k
                                for par in range(2):
                                    h = 2 * hp + par
                                    nc.tensor.matmul(
                                        pavs[par], vext_s[:, kpt, h, :],
                                        e_s[:, par, k, :],
                                        start=(kpt == 0), stop=(kpt == KPT - 1))
                            if PACE:
                                nc.tensor.matmul(
                                    pdum[:, 0:256], ident16,
                                    w2T_s.rearrange("p h t d -> p (h t d)")[:, 0:256],
                                    start=(pr == 0), stop=(pr == 7))
                        # drain pav fast: raw O rows + sums row, then normalize
                        for par in range(2):
                            h = 2 * hp + par
                            pb = 64 * par
                            pav = pavs[par]
                            oslice = ot_s[pb:pb + 64, dt_h, qc * 512:(qc + 1) * 512]
                            nc.vector.tensor_copy(out=oslice, in_=pav[0:HD, :])
                            srow = smallp.tile([1, 512], F32, tag="srow", bufs=1)
                            nc.vector.tensor_copy(out=srow, in_=pav[HD:HD + 1, :])
                            dtmp = drp.tile([1, 512], F32, tag="dtmp")
                            nc.sync.dma_start(out=dtmp, in_=srow)
                            rb = rbp.tile([128, 512], F32, tag="rb")
                            nc.sync.dma_start(
                                out=rb,
                                in_=bass.AP(tensor=dtmp.tensor, offset=dtmp.offset,
                                            ap=[[0, 128]] + list(dtmp.ap[1:])))
                            nc.vector.reciprocal_approx_fast(out=rb, in_=rb)
                            nc.vector.tensor_mul(out=oslice, in0=oslice,
                                                 in1=rb[pb:pb + 64, :])

                    # per-qc: O-projection + bias + residual + LN + transpose
                    def osrc(qt):
                        ps = ps_mm.tile([128, 512], F32, tag="mm")
                        for dt in range(KT):
                            nc.tensor.matmul(ps, ot_s[:, dt, qt * 128:(qt + 1) * 128],
                                             wts[:, dt, 3, :],
                                             start=(dt == 0), stop=(dt == KT - 1))
                        t = tpool.tile([128, D], F32, tag="t")
                        nc.vector.tensor_add(out=t, in0=ps, in1=resid_s[:, qt, :])
                        if (pre, "o") in bq_s:
                            nc.vector.tensor_add(out=t, in0=t, in1=bq_s[(pre, "o")])
                        return t

                    if PACE:
                        pdum3 = ps_mm.tile([128, 512], F32, tag="mm", name="pdum3")

                    def post(qt):
                        for dt in range(KT):
                            ptr = ps_mm.tile([128, 128], F32, tag="mm")
                            nc.tensor.transpose(
                                ptr, x_next[:, qt, dt * 128:(dt + 1) * 128], ident)
                            nc.vector.tensor_copy(
                                out=xnextT[:, dt, qt * 128:(qt + 1) * 128], in_=ptr)
                        if PACE:
                            nc.tensor.matmul(
                                pdum3[:, 0:256], ident16,
                                w2T_s.rearrange("p h t d -> p (h t d)")[:, 0:256],
                                start=(qt % 4 == 0), stop=(qt % 4 == 3))

                    layernorm_rows([qc * 4 + i for i in range(4)], lnidx, osrc,
                                   lambda qt: x_next[:, qt, :], post)
                return x_next, xnextT

            x1_s, x1T_s = attn_block("self", xkvT_s, 0, xkvT_s, xq_s, 1)
            x2_s, x2T_s = attn_block("cross", x1T_s, 0, ckvT_s, x1_s, 2)

            # ---- per-head FFN: head-pair packed, feature-major y,
            # transpose fused with residual accumulation into x2_s ----
            w1T_s = qkvp.tile([128, 4, FF], BF16, tag="kt", bufs=2)
            nc.sync.dma_start(out=w1T_s, in_=w1T_d.rearrange("hp p f -> p hp f"))
            for qc in range(QC):
                for hp in range(H // 2):
                    pyt = ps_av.tile([128, 512], F32, tag="av")
                    if PACE:
                        pdum2 = ps_mm.tile([128, 512], F32, tag="mm",
                                           name="pdum2")
                    for chunk in range(8):
                        hT = hpp.tile([128, 4, 512], BF16, tag="hT")
                        for f in range(2):
                            ft = chunk * 2 + f
                            for par in range(2):
                                h = 2 * hp + par
                                pb = 64 * par
                                ps = ps_mm.tile([128, 512], F32, tag="mm")
                                nc.tensor.matmul(
                                    ps, w1T_s[pb:pb + 64, hp, ft * 128:(ft + 1) * 128],
                                    x2T_s[pb:pb + 64, hp, qc * 512:(qc + 1) * 512],
                                    start=True, stop=True, tile_position=(pb, 0))
                                dst = hT[:, 2 * f + par, :]
                                if not flags["ffb1_zero"]:
                                    if par == 0:
                                        nc.scalar.activation(out=dst, in_=ps,
                                                             func=Relu,
                                                             bias=ffb1_s[:, h, ft:ft + 1])
                                    else:
                                        nc.vector.tensor_scalar(
                                            out=dst, in0=ps,
                                            scalar1=ffb1_s[:, h, ft:ft + 1],
                                            scalar2=0.0, op0=Add, op1=Max)
                                else:
                                    if par == 0:
                                        nc.scalar.activation(out=dst, in_=ps,
                                                             func=Relu)
                                    else:
                                        nc.vector.tensor_scalar_max(out=dst,
                                                                    in0=ps,
                                                                    scalar1=0.0)
                        # FFN2: col-packed pair, accumulate into pyt [128, 512]
                        for f in range(2):
                            ft = chunk * 2 + f
                            for par in range(2):
                                h = 2 * hp + par
                                nc.tensor.matmul(
                                    pyt[64 * par:64 * par + 64, :],
                                    w2T_s[:, h, ft, :],
                                    hT[:, 2 * f + par, :],
                                    start=(ft == 0), stop=(ft == FT - 1),
                                    tile_position=(0, 64 * par))
                        if PACE:
                            nc.tensor.matmul(
                                pdum2[:, 0:256], ident16,
                                w1T_s.rearrange("p a b -> p (a b)")[:, 0:256].bitcast(BF16),
                                start=(chunk == 0), stop=(chunk == 7))

                    # drain yT pair: transpose + residual-accumulate into x2_s
                    yT = hpp.tile([128, 512], BF16, tag="yT", bufs=2)
                    if PACE:
                        pass
                    nc.vector.tensor_copy(out=yT, in_=pyt)
                    for qt4 in range(4):
                        qt = qc * 4 + qt4
                        ptr = ps_mm.tile([128, 128], BF16, tag="mm")
                        nc.tensor.transpose(
                            ptr, yT[:, qt4 * 128:(qt4 + 1) * 128], ident16)
                        sl = x2_s[:, qt, hp * 128:(hp + 1) * 128]
                        nc.vector.tensor_add(out=sl, in0=sl, in1=ptr)

            # ---- LN3 + output ----
            def src3(qt):
                if not flags["ffb2_zero"]:
                    nc.vector.tensor_add(out=x2_s[:, qt, :], in0=x2_s[:, qt, :],
                                         in1=ffb2_s)
                return x2_s[:, qt, :]

            out_r = out_d.rearrange("(t p) d -> p t d", p=128)
            for g0 in (0, 4):
                outs = {}

                def dst3(qt):
                    o = tpool.tile([128, D], F32, tag="t")
                    outs[qt] = o
                    return o

                def post3(qt):
                    nc.sync.dma_start(out=out_r[:, qt, :], in_=outs[qt])

                layernorm_rows([g0 + i for i in range(4)], 3, src3, dst3, post3)

    nc.compile()
    return nc


def kernel(x, cross, params):
    x = np.asarray(x, dtype=np.float32)
    cross = np.asarray(cross, dtype=np.float32)
    p = {k: np.asarray(v) for k, v in params.items()}

    flags = {}
    for pre in ("self", "cross"):
        for nm in ("q", "k", "v", "o"):
            flags[f"{pre}_b{nm}_zero"] = not np.any(p[f"{pre}_b{nm}"])
    for i in (1, 2, 3):
        flags[f"ln{i}_id"] = (np.all(p[f"ln{i}_g"] == 1.0)
                              and not np.any(p[f"ln{i}_b"]))
    flags["ffb1_zero"] = not np.any(p["ff_b1"])
    flags["ffb2_zero"] = not np.any(p["ff_b2"])

    key = tuple(sorted(flags.items()))
    if key not in _CACHE:
        _CACHE[key] = _build(flags)
    nc = _CACHE[key]

    def wpack(pre):
        mats = []
        for nm, scale in (("q", 0.125), ("k", 1.0), ("v", 1.0), ("o", 1.0)):
            mats.append((p[f"{pre}_w{nm}"].T * scale).astype(NP_BF16))
        return np.ascontiguousarray(np.stack(mats, axis=1))  # [D, 4, D]

    shared = {
        "self_w": wpack("self"),
        "cross_w": wpack("cross"),
        "w1T": np.ascontiguousarray(
            p["ff_w1"].transpose(0, 2, 1).reshape(4, 128, FF)).astype(NP_BF16),
        "w2T": np.ascontiguousarray(p["ff_w2"].transpose(0, 2, 1)).astype(NP_BF16),
    }
    for pre in ("self", "cross"):
        for nm in ("q", "k", "v", "o"):
            if not flags[f"{pre}_b{nm}_zero"]:
                v = p[f"{pre}_b{nm}"].astype(np.float32)
                if nm == "q":
                    v = v * 0.125
                shared[f"{pre}_b{nm}"] = v
    for i in (1, 2, 3):
        if not flags[f"ln{i}_id"]:
            shared[f"ln{i}_g"] = p[f"ln{i}_g"].astype(np.float32)
            shared[f"ln{i}_b"] = p[f"ln{i}_b"].astype(np.float32)
    if not flags["ffb1_zero"]:
        shared["ffb1"] = p["ff_b1"].astype(np.float32)
    if not flags["ffb2_zero"]:
        shared["ffb2"] = p["ff_b2"].reshape(D).astype(np.float32)

    in_maps = []
    ckvT_cache = {}
    for c in range(NCORES):
        b, half = c // 2, c % 2
        xr = np.roll(x[b], -R * half, axis=0)
        if b not in ckvT_cache:
            ckvT_cache[b] = np.ascontiguousarray(cross[b].T).astype(NP_BF16)
        m = dict(shared)
        m["xq"] = np.ascontiguousarray(xr[:R])
        m["xkvT"] = np.ascontiguousarray(xr.T).astype(NP_BF16)
        m["ckvT"] = ckvT_cache[b]
        in_maps.append(m)

    res = run_bass_kernel_spmd(nc, in_maps, core_ids=list(range(NCORES)))
    kernel._last_results = res

    y = np.empty((B, L, D), np.float32)
    for c in range(NCORES):
        b, half = c // 2, c % 2
        y[b, R * half:R * (half + 1)] = res.results[c]["out"]
    return y
